# revision 30
# baseline (speedup 1.0000x reference)
"""Multi-head causal attention (B=2, T=2048, E=1024, H=16, D=64) on 8 trn2
NeuronCores.

Sharding: tensor-parallel over heads. Core c owns heads {2c, 2c+1} (128 of the
1024 qkv channels). Each core computes its heads' QKV projections, attention,
and a row-parallel slice of the output projection, producing a partial [4096,
1024] output. The host sums the 8 partials and adds b_proj.

On-chip layout is transpose-free: the host supplies x^T, the kernel computes
Q^T/K^T/V^T [c, t] directly (weights as stationary operand), attention scores
are built transposed (S^T = K Q^T, [k, q]) so that P^T feeds the PV matmul with
V in natural [t, c] layout (recovered with a PE transpose), which in turn
yields Y^T [c, t] — exactly the stationary operand the output projection needs.
Softmax denominators ride along as an extra ones-column in the PV stationary
operand; normalization uses a K=1 ones-matmul to broadcast 1/denom across
partitions.
"""

import os
import sys

import numpy as np

for _p in ("/opt/trn_rl_repo", "/root/.axon_site/_ro/trn_rl_repo"):
    if os.path.isdir(_p) and _p not in sys.path:
        sys.path.append(_p)

B, T, E, H, D = 2, 2048, 1024, 16, 64
NC_CORES = 8
TT = B * T            # 4096 total tokens
EC = E // 128         # 8 contraction chunks
TG = TT // 512        # 8 token groups for the QKV pass
QC = T // 512         # 4 query chunks per batch
KB = T // 128         # 16 key blocks per batch

_last_results = None   # test harness peeks at this for exec_time_ns


def _build_bass(mm_dtype="float32r"):
    import concourse.bacc as bacc
    import concourse.mybir as mybir
    import concourse.tile as tile
    from concourse.masks import make_identity

    FP = mybir.dt.float32
    # Dtype of every tensor feeding a matmul. float32r = fp32 storage with
    # PE-rounded (tf32-like) multiplies at ~4x the fp32 matmul rate; walrus
    # requires producers of matmul operands to round to it, so the whole feed
    # path is tagged. float32 gives exact (slow) matmuls with the same graph.
    MF = getattr(mybir.dt, mm_dtype)
    AF = mybir.ActivationFunctionType

    nc = bacc.Bacc("TRN2", debug=False)
    xt = nc.declare_dram_parameter("xt", [EC, 128, TT], MF, isOutput=False)
    wqkv = nc.declare_dram_parameter("wqkv", [128, EC, 384], MF, isOutput=False)
    bqkv = nc.declare_dram_parameter("bqkv", [128, 3], FP, isOutput=False)
    wproj = nc.declare_dram_parameter("wproj", [128, E], MF, isOutput=False)
    y = nc.declare_dram_parameter("y", [TT, E], FP, isOutput=True)

    with tile.TileContext(nc) as tc:
        with tc.tile_pool(name="big", bufs=1) as big, \
             tc.tile_pool(name="const", bufs=1) as const:
            # weights/bias on the gpsimd DMA queue so the x-chunk loads own
            # the sync queue from instruction 0
            w_sb = const.tile([128, EC, 384], MF, tag="w_sb")
            for e in range(EC):
                nc.gpsimd.dma_start(out=w_sb[:, e, :], in_=wqkv[:, e, :])
            b_sb = const.tile([128, 3], FP, tag="b_sb")
            nc.gpsimd.dma_start(out=b_sb, in_=bqkv[:, :])
            wp_sb = const.tile([128, E], MF, tag="wp_sb")
            nc.gpsimd.dma_start(out=wp_sb, in_=wproj[:, :])
            ident32 = const.tile([128, 128], FP, tag="ident32")
            make_identity(nc, ident32)
            ident = const.tile([128, 128], MF, tag="ident")
            nc.vector.tensor_copy(ident, ident32)
            # fp32 image of one V_aug block's cols 64:192 (all denominator ones)
            vpat = const.tile([128, 128], FP, tag="vpat")
            nc.gpsimd.memset(vpat, 1.0)
            # tri[k, q] = 1.0 where q >= k else 0.0  (keep-mask applied after exp)
            trimask = const.tile([128, 128], FP, tag="trimask")
            nc.gpsimd.memset(trimask, 1.0)
            nc.gpsimd.affine_select(
                out=trimask, in_=trimask,
                compare_op=mybir.AluOpType.is_ge,
                fill=0.0, base=0,
                pattern=[[1, 128]], channel_multiplier=-1,
            )

            qt_sb = big.tile([128, TT], MF, tag="qt")    # [c(2 heads x 64d), t]
            kt_sb = big.tile([128, TT], MF, tag="kt")
            vt_sb = big.tile([128, TT], MF, tag="vt")
            # V_aug blocks, per 128-token block j:
            #   cols   0: 64  V_h0      cols  64:128 ones
            #   cols 128:192 ones       cols 192:256 V_h1
            # lhsT h0 = [:, j, 0:128]  -> psum rows 0:64 Y_h0, rows 64:128 denom_h0
            # lhsT h1 = [:, j, 128:256]-> psum rows 0:64 denom_h1, rows 64:128 Y_h1
            vaug = big.tile([128, KB * B, 256], MF, tag="vaug")
            yt_sb = big.tile([128, TT], MF, tag="yt")

            # ---- Phase 1: Q^T / K^T / V^T = W.T @ x^T (+bias) ----
            with tc.tile_pool(name="xin", bufs=8) as xin, \
                 tc.tile_pool(name="qkv_ps", bufs=2, space="PSUM") as qkv_ps:
                for tg in range(TG):
                    ts = slice(tg * 512, (tg + 1) * 512)
                    pq = qkv_ps.tile([128, 512], FP, tag="pq")
                    pk = qkv_ps.tile([128, 512], FP, tag="pk")
                    pv = qkv_ps.tile([128, 512], FP, tag="pv")
                    for e in range(EC):
                        xc = xin.tile([128, 512], MF, tag="xc")
                        nc.sync.dma_start(out=xc, in_=xt[e, :, ts])
                        st_, sp_ = (e == 0), (e == EC - 1)
                        nc.tensor.matmul(pq, w_sb[:, e, 0:128], xc, start=st_, stop=sp_)
                        nc.tensor.matmul(pk, w_sb[:, e, 128:256], xc, start=st_, stop=sp_)
                        nc.tensor.matmul(pv, w_sb[:, e, 256:384], xc, start=st_, stop=sp_)
                    nc.scalar.activation(qt_sb[:, ts], pq, AF.Identity, bias=b_sb[:, 0:1])
                    nc.scalar.activation(kt_sb[:, ts], pk, AF.Identity, bias=b_sb[:, 1:2])
                    nc.scalar.activation(vt_sb[:, ts], pv, AF.Identity, bias=b_sb[:, 2:3])

            # ---- Phase 1b: V_aug = transpose(V^T) blocks + ones columns ----
            with tc.tile_pool(name="tp_ps", bufs=2, space="PSUM") as tp_ps:
                for j in range(KB * B):
                    tp = tp_ps.tile([128, 128], MF, tag="tp")
                    nc.tensor.transpose(tp, vt_sb[:, j * 128:(j + 1) * 128], ident)
                    nc.vector.tensor_copy(vaug[:, j, 64:192], vpat)
                    nc.vector.tensor_copy(vaug[:, j, 0:64], tp[:, 0:64])
                    nc.vector.tensor_copy(vaug[:, j, 192:256], tp[:, 64:128])

            # ---- Phase 2: attention, per batch, both heads ----
            with tc.tile_pool(name="att_ps", bufs=1, space="PSUM") as att_ps, \
                 tc.tile_pool(name="st_ps", bufs=2, space="PSUM") as st_ps, \
                 tc.tile_pool(name="pt_sb", bufs=4) as pt_pool, \
                 tc.tile_pool(name="out_sb", bufs=4) as out_pool, \
                 tc.tile_pool(name="rc_sb", bufs=2) as rc_pool:
                for b in range(B):
                    for qc in range(QC):
                        yt0 = att_ps.tile([128, 512], FP, tag="yt0", bufs=2)
                        yt1 = att_ps.tile([128, 512], FP, tag="yt1", bufs=2)
                        njb = 4 * qc + 4
                        for j in range(njb):
                            qoff = max(512 * qc, 128 * j)
                            width = 512 * qc + 512 - qoff
                            st = st_ps.tile([128, 2, 512], FP, tag="st")
                            q0 = b * T + qoff
                            k0 = b * T + 128 * j
                            nc.tensor.matmul(
                                st[:, 0, 0:width],
                                kt_sb[0:64, k0:k0 + 128],
                                qt_sb[0:64, q0:q0 + width],
                                start=True, stop=True)
                            nc.tensor.matmul(
                                st[:, 1, 0:width],
                                kt_sb[64:128, k0:k0 + 128],
                                qt_sb[64:128, q0:q0 + width],
                                start=True, stop=True)
                            pt = pt_pool.tile([128, 2, 512], MF, tag="pt")
                            nc.scalar.activation(
                                pt[:, :, 0:width], st[:, :, 0:width],
                                AF.Exp, scale=0.125)
                            if j >= 4 * qc:  # diagonal block: causal mask
                                nc.vector.tensor_mul(pt[:, 0, 0:128], pt[:, 0, 0:128], trimask)
                                nc.vector.tensor_mul(pt[:, 1, 0:128], pt[:, 1, 0:128], trimask)
                            off = qoff - 512 * qc
                            jb = b * KB + j
                            st_, sp_ = (j == 0), (j == njb - 1)
                            nc.tensor.matmul(
                                yt0[:, off:512], vaug[:, jb, 0:128],
                                pt[:, 0, 0:width], start=st_, stop=sp_)
                            nc.tensor.matmul(
                                yt1[:, off:512], vaug[:, jb, 128:256],
                                pt[:, 1, 0:width], start=st_, stop=sp_)
                        # normalize: yt0 = [Y_h0 (0:64) ; denom_h0 rep (64:128)]
                        #            yt1 = [denom_h1 rep (0:64) ; Y_h1 (64:128)]
                        # stream_shuffle moves the denom halves onto the same
                        # partitions as their Y half, then one recip + two muls.
                        iden_mask = list(range(32))
                        den_sb = rc_pool.tile([128, 512], FP, tag="den_sb")
                        nc.vector.stream_shuffle(den_sb[0:64, :], yt0[64:128, :], iden_mask)
                        nc.vector.stream_shuffle(den_sb[64:128, :], yt1[0:64, :], iden_mask)
                        rcb_sb = rc_pool.tile([128, 512], FP, tag="rcb_sb")
                        rscr = rc_pool.tile([128, 512], FP, tag="rscr")
                        nc.vector.reciprocal_approx_accurate(rcb_sb, den_sb, rscr)
                        cs = slice(b * T + 512 * qc, b * T + 512 * qc + 512)
                        nc.vector.tensor_mul(yt_sb[0:64, cs], yt0[0:64, :], rcb_sb[0:64, :])
                        nc.vector.tensor_mul(yt_sb[64:128, cs], yt1[64:128, :], rcb_sb[64:128, :])
                        # output projection for this q-chunk's 4 token blocks,
                        # inlined so the PE never has a cold tail phase
                        for tb in range(4 * (b * QC + qc), 4 * (b * QC + qc) + 4):
                            for ecol in range(E // 512):
                                pp = st_ps.tile([128, 512], FP, tag="st")
                                nc.tensor.matmul(
                                    pp, yt_sb[:, tb * 128:(tb + 1) * 128],
                                    wp_sb[:, ecol * 512:(ecol + 1) * 512],
                                    start=True, stop=True)
                                ot = out_pool.tile([128, 512], FP, tag="ot")
                                nc.vector.tensor_copy(ot, pp)
                                nc.sync.dma_start(
                                    out=y[tb * 128:(tb + 1) * 128,
                                          ecol * 512:(ecol + 1) * 512],
                                    in_=ot)

    nc.compile()
    return nc


_nc_cache = {}


def get_nc(mm_dtype="float32r"):
    if mm_dtype not in _nc_cache:
        _nc_cache[mm_dtype] = _build_bass(mm_dtype)
    return _nc_cache[mm_dtype]


def make_in_maps(x, W_attn, b_attn, W_proj, feed_dtype=np.float32):
    x = np.ascontiguousarray(np.asarray(x, np.float32))
    W_attn = np.asarray(W_attn, np.float32)
    b_attn = np.asarray(b_attn, np.float32)
    W_proj = np.asarray(W_proj, np.float32)
    xtf = np.ascontiguousarray(x.reshape(TT, E).T.astype(feed_dtype)).reshape(EC, 128, TT)
    in_maps = []
    for c in range(NC_CORES):
        s = slice(128 * c, 128 * c + 128)
        wq = W_attn[:, 128 * c:128 * c + 128]
        wk = W_attn[:, E + 128 * c:E + 128 * c + 128]
        wv = W_attn[:, 2 * E + 128 * c:2 * E + 128 * c + 128]
        wqkv = np.concatenate([wq, wk, wv], axis=1)          # [E, 384]
        wqkv = np.ascontiguousarray(
            wqkv.reshape(EC, 128, 384).transpose(1, 0, 2).astype(feed_dtype))
        bq = b_attn[128 * c:128 * c + 128]
        bk = b_attn[E + 128 * c:E + 128 * c + 128]
        bv = b_attn[2 * E + 128 * c:2 * E + 128 * c + 128]
        bqkv = np.ascontiguousarray(np.stack([bq, bk, bv], axis=1))  # [128, 3]
        wp = np.ascontiguousarray(W_proj[s, :].astype(feed_dtype))   # [128, E]
        in_maps.append({"xt": xtf, "wqkv": wqkv, "bqkv": bqkv, "wproj": wp})
    return in_maps


def _install_ntff_shim():
    """The image's antenv package lacks axon_hooks, which bass_utils imports
    when trace=True under axon. Install it at runtime (registering the same
    ctypes-based hook trn_agent_boot would have set)."""
    import types
    import antenv
    if hasattr(antenv, "axon_hooks"):
        return
    mod = types.ModuleType("antenv.axon_hooks")
    mod._hook = None
    mod.set_axon_ntff_profile_hook = lambda h: setattr(mod, "_hook", h)
    mod.get_axon_ntff_profile_hook = lambda: mod._hook
    sys.modules["antenv.axon_hooks"] = mod
    antenv.axon_hooks = mod
    try:
        from trn_agent_boot.trn_boot import _ntff_profile_via_ctypes
        mod._hook = _ntff_profile_via_ctypes("/opt/axon/libaxon_pjrt.so")
    except Exception as e:
        print("ntff shim: no hook:", e)
    # upload_artifacts reaches external blob storage; stub it out.
    from concourse import bass_utils as bu
    bu.upload_artifacts = lambda tmpdir: tmpdir


def kernel(x, W_attn, b_attn, W_proj, b_proj):
    global _last_results
    from concourse.bass_utils import run_bass_kernel_spmd

    trace_req = bool(int(os.environ.get("KERNEL_TRACE", "0")))
    if trace_req:
        _install_ntff_shim()
    mm_dtype = os.environ.get("KERNEL_MM_DTYPE", "float32r")
    if mm_dtype == "bfloat16":
        import ml_dtypes
        feed_dtype = ml_dtypes.bfloat16
    else:
        feed_dtype = np.float32
    nc = get_nc(mm_dtype)
    in_maps = make_in_maps(x, W_attn, b_attn, W_proj, feed_dtype)
    res = run_bass_kernel_spmd(
        nc, in_maps, core_ids=list(range(NC_CORES)), trace=trace_req)
    _last_results = res
    y = res.results[0]["y"].astype(np.float64)
    for r in res.results[1:]:
        y += r["y"]
    y += np.asarray(b_proj, np.float64)
    return y.astype(np.float32).reshape(B, T, E)


# revision 33
# speedup vs baseline: 1.1038x; 1.1038x over previous
"""Multi-head causal attention (B=2, T=2048, E=1024, H=16, D=64) on 8 trn2
NeuronCores.

Sharding: tensor-parallel over heads. Core c owns heads {2c, 2c+1} (128 of the
1024 qkv channels). Each core computes its heads' QKV projections, attention,
and a row-parallel slice of the output projection, producing a partial [4096,
1024] output. The host sums the 8 partials and adds b_proj.

On-chip layout is transpose-free: the host supplies x^T, the kernel computes
Q^T/K^T/V^T [c, t] directly (weights as stationary operand), attention scores
are built transposed (S^T = K Q^T, [k, q]) so that P^T feeds the PV matmul with
V in natural [t, c] layout (recovered with a PE transpose), which in turn
yields Y^T [c, t] — exactly the stationary operand the output projection needs.
Softmax denominators ride along as an extra ones-column in the PV stationary
operand; normalization uses a K=1 ones-matmul to broadcast 1/denom across
partitions.
"""

import os
import sys

import numpy as np

for _p in ("/opt/trn_rl_repo", "/root/.axon_site/_ro/trn_rl_repo"):
    if os.path.isdir(_p) and _p not in sys.path:
        sys.path.append(_p)

B, T, E, H, D = 2, 2048, 1024, 16, 64
NC_CORES = 8
TT = B * T            # 4096 total tokens
EC = E // 128         # 8 contraction chunks
TG = TT // 512        # 8 token groups for the QKV pass
QC = T // 512         # 4 query chunks per batch
KB = T // 128         # 16 key blocks per batch

_last_results = None   # test harness peeks at this for exec_time_ns


def _build_bass(mm_dtype="float32r"):
    import concourse.bacc as bacc
    import concourse.mybir as mybir
    import concourse.tile as tile
    from concourse.masks import make_identity

    FP = mybir.dt.float32
    # Dtype of every tensor feeding a matmul. float32r = fp32 storage with
    # PE-rounded (tf32-like) multiplies at ~4x the fp32 matmul rate; walrus
    # requires producers of matmul operands to round to it, so the whole feed
    # path is tagged. float32 gives exact (slow) matmuls with the same graph.
    MF = getattr(mybir.dt, mm_dtype)
    AF = mybir.ActivationFunctionType

    nc = bacc.Bacc("TRN2", debug=False)
    xt = nc.declare_dram_parameter("xt", [EC, 128, TT], MF, isOutput=False)
    wqkv = nc.declare_dram_parameter("wqkv", [128, EC, 384], MF, isOutput=False)
    bqkv = nc.declare_dram_parameter("bqkv", [128, 3], FP, isOutput=False)
    wproj = nc.declare_dram_parameter("wproj", [128, E], MF, isOutput=False)
    y = nc.declare_dram_parameter("y", [TT, E], FP, isOutput=True)

    with tile.TileContext(nc) as tc:
        with tc.tile_pool(name="big", bufs=1) as big, \
             tc.tile_pool(name="const", bufs=1) as const:
            # weights/bias on the gpsimd DMA queue so the x-chunk loads own
            # the sync queue from instruction 0
            w_sb = const.tile([128, EC, 384], MF, tag="w_sb")
            for e in range(EC):
                nc.gpsimd.dma_start(out=w_sb[:, e, :], in_=wqkv[:, e, :])
            b_sb = const.tile([128, 3], FP, tag="b_sb")
            nc.gpsimd.dma_start(out=b_sb, in_=bqkv[:, :])
            wp_sb = const.tile([128, E], MF, tag="wp_sb")
            nc.gpsimd.dma_start(out=wp_sb, in_=wproj[:, :])
            ident32 = const.tile([128, 128], FP, tag="ident32")
            make_identity(nc, ident32)
            ident = const.tile([128, 128], MF, tag="ident")
            nc.vector.tensor_copy(ident, ident32)
            # fp32 image of one V_aug block's cols 64:192 (all denominator ones)
            vpat = const.tile([128, 128], FP, tag="vpat")
            nc.gpsimd.memset(vpat, 1.0)
            # tri[k, q] = 1.0 where q >= k else 0.0  (keep-mask applied after exp)
            trimask = const.tile([128, 128], FP, tag="trimask")
            nc.gpsimd.memset(trimask, 1.0)
            nc.gpsimd.affine_select(
                out=trimask, in_=trimask,
                compare_op=mybir.AluOpType.is_ge,
                fill=0.0, base=0,
                pattern=[[1, 128]], channel_multiplier=-1,
            )

            qt_sb = big.tile([128, TT], MF, tag="qt")    # [c(2 heads x 64d), t]
            kt_sb = big.tile([128, TT], MF, tag="kt")
            vt_sb = big.tile([128, TT], MF, tag="vt")
            # V_aug blocks, per 128-token block j:
            #   cols   0: 64  V_h0      cols  64:128 ones
            #   cols 128:192 ones       cols 192:256 V_h1
            # lhsT h0 = [:, j, 0:128]  -> psum rows 0:64 Y_h0, rows 64:128 denom_h0
            # lhsT h1 = [:, j, 128:256]-> psum rows 0:64 denom_h1, rows 64:128 Y_h1
            vaug = big.tile([128, KB * B, 256], MF, tag="vaug")
            yt_sb = big.tile([128, TT], MF, tag="yt")

            # ---- Phase 1: Q^T / K^T / V^T = W.T @ x^T (+bias), with the
            # V-block PE-transposes interleaved per token group (a standalone
            # transpose pass doesn't count as PE-busy for the HAM clock gate
            # and lets it throttle right before the attention phase) ----
            with tc.tile_pool(name="xin", bufs=8) as xin, \
                 tc.tile_pool(name="qkv_ps", bufs=2, space="PSUM") as qkv_ps, \
                 tc.tile_pool(name="tp_ps", bufs=2, space="PSUM") as tp_ps:
                for tg in range(TG):
                    ts = slice(tg * 512, (tg + 1) * 512)
                    pq = qkv_ps.tile([128, 512], FP, tag="pq")
                    pk = qkv_ps.tile([128, 512], FP, tag="pk")
                    pv = qkv_ps.tile([128, 512], FP, tag="pv")
                    for e in range(EC):
                        xc = xin.tile([128, 512], MF, tag="xc")
                        nc.sync.dma_start(out=xc, in_=xt[e, :, ts])
                        st_, sp_ = (e == 0), (e == EC - 1)
                        nc.tensor.matmul(pq, w_sb[:, e, 0:128], xc, start=st_, stop=sp_)
                        nc.tensor.matmul(pk, w_sb[:, e, 128:256], xc, start=st_, stop=sp_)
                        nc.tensor.matmul(pv, w_sb[:, e, 256:384], xc, start=st_, stop=sp_)
                    nc.scalar.activation(qt_sb[:, ts], pq, AF.Identity, bias=b_sb[:, 0:1])
                    nc.scalar.activation(kt_sb[:, ts], pk, AF.Identity, bias=b_sb[:, 1:2])
                    nc.scalar.activation(vt_sb[:, ts], pv, AF.Identity, bias=b_sb[:, 2:3])
                    if tg:  # transpose the PREVIOUS group's V blocks (hides
                        # the ACT->PE latency behind this group's matmuls)
                        for j in range(4 * (tg - 1), 4 * tg):
                            tp = tp_ps.tile([128, 128], MF, tag="tp")
                            nc.tensor.transpose(tp, vt_sb[:, j * 128:(j + 1) * 128], ident)
                            nc.vector.tensor_copy(vaug[:, j, 64:192], vpat)
                            nc.vector.tensor_copy(vaug[:, j, 0:64], tp[:, 0:64])
                            nc.vector.tensor_copy(vaug[:, j, 192:256], tp[:, 64:128])
                for j in range(4 * (TG - 1), 4 * TG):
                    tp = tp_ps.tile([128, 128], MF, tag="tp")
                    nc.tensor.transpose(tp, vt_sb[:, j * 128:(j + 1) * 128], ident)
                    nc.vector.tensor_copy(vaug[:, j, 64:192], vpat)
                    nc.vector.tensor_copy(vaug[:, j, 0:64], tp[:, 0:64])
                    nc.vector.tensor_copy(vaug[:, j, 192:256], tp[:, 64:128])

            # ---- Phase 2: attention, per batch, both heads ----
            with tc.tile_pool(name="att_ps", bufs=1, space="PSUM") as att_ps, \
                 tc.tile_pool(name="st_ps", bufs=2, space="PSUM") as st_ps, \
                 tc.tile_pool(name="pt_sb", bufs=4) as pt_pool, \
                 tc.tile_pool(name="rc_sb", bufs=2) as rc_pool:
                for b in range(B):
                    for qc in range(QC):
                        yt0 = att_ps.tile([128, 512], FP, tag="yt0", bufs=2)
                        yt1 = att_ps.tile([128, 512], FP, tag="yt1", bufs=2)
                        njb = 4 * qc + 4
                        for j in range(njb):
                            qoff = max(512 * qc, 128 * j)
                            width = 512 * qc + 512 - qoff
                            st = st_ps.tile([128, 2, 512], FP, tag="st")
                            q0 = b * T + qoff
                            k0 = b * T + 128 * j
                            nc.tensor.matmul(
                                st[:, 0, 0:width],
                                kt_sb[0:64, k0:k0 + 128],
                                qt_sb[0:64, q0:q0 + width],
                                start=True, stop=True)
                            nc.tensor.matmul(
                                st[:, 1, 0:width],
                                kt_sb[64:128, k0:k0 + 128],
                                qt_sb[64:128, q0:q0 + width],
                                start=True, stop=True)
                            pt = pt_pool.tile([128, 2, 512], MF, tag="pt")
                            nc.scalar.activation(
                                pt[:, :, 0:width], st[:, :, 0:width],
                                AF.Exp, scale=0.125)
                            if j >= 4 * qc:  # diagonal block: causal mask
                                nc.vector.tensor_mul(pt[:, 0, 0:128], pt[:, 0, 0:128], trimask)
                                nc.vector.tensor_mul(pt[:, 1, 0:128], pt[:, 1, 0:128], trimask)
                            off = qoff - 512 * qc
                            jb = b * KB + j
                            st_, sp_ = (j == 0), (j == njb - 1)
                            nc.tensor.matmul(
                                yt0[:, off:512], vaug[:, jb, 0:128],
                                pt[:, 0, 0:width], start=st_, stop=sp_)
                            nc.tensor.matmul(
                                yt1[:, off:512], vaug[:, jb, 128:256],
                                pt[:, 1, 0:width], start=st_, stop=sp_)
                        # normalize: yt0 = [Y_h0 (0:64) ; denom_h0 rep (64:128)]
                        #            yt1 = [denom_h1 rep (0:64) ; Y_h1 (64:128)]
                        # stream_shuffle moves the denom halves onto the same
                        # partitions as their Y half, then one recip + two muls.
                        iden_mask = list(range(32))
                        den_sb = rc_pool.tile([128, 512], FP, tag="den_sb")
                        nc.vector.stream_shuffle(den_sb[0:64, :], yt0[64:128, :], iden_mask)
                        nc.vector.stream_shuffle(den_sb[64:128, :], yt1[0:64, :], iden_mask)
                        rcb_sb = rc_pool.tile([128, 512], FP, tag="rcb_sb")
                        rscr = rc_pool.tile([128, 512], FP, tag="rscr")
                        nc.vector.reciprocal_approx_accurate(rcb_sb, den_sb, rscr)
                        cs = slice(b * T + 512 * qc, b * T + 512 * qc + 512)
                        nc.vector.tensor_mul(yt_sb[0:64, cs], yt0[0:64, :], rcb_sb[0:64, :])
                        nc.vector.tensor_mul(yt_sb[64:128, cs], yt1[64:128, :], rcb_sb[64:128, :])

            # ---- Phase 3: partial output projection ----
            with tc.tile_pool(name="pj_ps", bufs=4, space="PSUM") as pj_ps, \
                 tc.tile_pool(name="out_sb", bufs=4) as out_pool2:
                for tb in range(TT // 128):
                    for ecol in range(E // 512):
                        pp = pj_ps.tile([128, 512], FP, tag="pp")
                        nc.tensor.matmul(
                            pp, yt_sb[:, tb * 128:(tb + 1) * 128],
                            wp_sb[:, ecol * 512:(ecol + 1) * 512],
                            start=True, stop=True)
                        ot = out_pool2.tile([128, 512], FP, tag="ot")
                        nc.vector.tensor_copy(ot, pp)
                        nc.sync.dma_start(
                            out=y[tb * 128:(tb + 1) * 128, ecol * 512:(ecol + 1) * 512],
                            in_=ot)

    nc.compile()
    return nc


_nc_cache = {}


def get_nc(mm_dtype="float32r"):
    if mm_dtype not in _nc_cache:
        _nc_cache[mm_dtype] = _build_bass(mm_dtype)
    return _nc_cache[mm_dtype]


def make_in_maps(x, W_attn, b_attn, W_proj, feed_dtype=np.float32):
    x = np.ascontiguousarray(np.asarray(x, np.float32))
    W_attn = np.asarray(W_attn, np.float32)
    b_attn = np.asarray(b_attn, np.float32)
    W_proj = np.asarray(W_proj, np.float32)
    xtf = np.ascontiguousarray(x.reshape(TT, E).T.astype(feed_dtype)).reshape(EC, 128, TT)
    in_maps = []
    for c in range(NC_CORES):
        s = slice(128 * c, 128 * c + 128)
        wq = W_attn[:, 128 * c:128 * c + 128]
        wk = W_attn[:, E + 128 * c:E + 128 * c + 128]
        wv = W_attn[:, 2 * E + 128 * c:2 * E + 128 * c + 128]
        wqkv = np.concatenate([wq, wk, wv], axis=1)          # [E, 384]
        wqkv = np.ascontiguousarray(
            wqkv.reshape(EC, 128, 384).transpose(1, 0, 2).astype(feed_dtype))
        bq = b_attn[128 * c:128 * c + 128]
        bk = b_attn[E + 128 * c:E + 128 * c + 128]
        bv = b_attn[2 * E + 128 * c:2 * E + 128 * c + 128]
        bqkv = np.ascontiguousarray(np.stack([bq, bk, bv], axis=1))  # [128, 3]
        wp = np.ascontiguousarray(W_proj[s, :].astype(feed_dtype))   # [128, E]
        in_maps.append({"xt": xtf, "wqkv": wqkv, "bqkv": bqkv, "wproj": wp})
    return in_maps


def _install_ntff_shim():
    """The image's antenv package lacks axon_hooks, which bass_utils imports
    when trace=True under axon. Install it at runtime (registering the same
    ctypes-based hook trn_agent_boot would have set)."""
    import types
    import antenv
    if hasattr(antenv, "axon_hooks"):
        return
    mod = types.ModuleType("antenv.axon_hooks")
    mod._hook = None
    mod.set_axon_ntff_profile_hook = lambda h: setattr(mod, "_hook", h)
    mod.get_axon_ntff_profile_hook = lambda: mod._hook
    sys.modules["antenv.axon_hooks"] = mod
    antenv.axon_hooks = mod
    try:
        from trn_agent_boot.trn_boot import _ntff_profile_via_ctypes
        mod._hook = _ntff_profile_via_ctypes("/opt/axon/libaxon_pjrt.so")
    except Exception as e:
        print("ntff shim: no hook:", e)
    # upload_artifacts reaches external blob storage; stub it out.
    from concourse import bass_utils as bu
    bu.upload_artifacts = lambda tmpdir: tmpdir


def kernel(x, W_attn, b_attn, W_proj, b_proj):
    global _last_results
    from concourse.bass_utils import run_bass_kernel_spmd

    trace_req = bool(int(os.environ.get("KERNEL_TRACE", "0")))
    if trace_req:
        _install_ntff_shim()
    mm_dtype = os.environ.get("KERNEL_MM_DTYPE", "float32r")
    if mm_dtype == "bfloat16":
        import ml_dtypes
        feed_dtype = ml_dtypes.bfloat16
    else:
        feed_dtype = np.float32
    nc = get_nc(mm_dtype)
    in_maps = make_in_maps(x, W_attn, b_attn, W_proj, feed_dtype)
    res = run_bass_kernel_spmd(
        nc, in_maps, core_ids=list(range(NC_CORES)), trace=trace_req)
    _last_results = res
    y = res.results[0]["y"].astype(np.float64)
    for r in res.results[1:]:
        y += r["y"]
    y += np.asarray(b_proj, np.float64)
    return y.astype(np.float32).reshape(B, T, E)


# revision 36
# speedup vs baseline: 1.4914x; 1.3512x over previous
"""Multi-head causal attention (B=2, T=2048, E=1024, H=16, D=64) on 8 trn2
NeuronCores.

Sharding: tensor-parallel over heads. Core c owns heads {2c, 2c+1} (128 of the
1024 qkv channels). Each core computes its heads' QKV projections, attention,
and a row-parallel slice of the output projection, producing a partial [4096,
1024] output. The host sums the 8 partials and adds b_proj.

On-chip layout is transpose-free: the host supplies x^T, the kernel computes
Q^T/K^T/V^T [c, t] directly (weights as stationary operand), attention scores
are built transposed (S^T = K Q^T, [k, q]) so that P^T feeds the PV matmul with
V in natural [t, c] layout (recovered with a PE transpose), which in turn
yields Y^T [c, t] — exactly the stationary operand the output projection needs.
Softmax denominators ride along as 64 replicated ones-columns in the PV
stationary operand; a DVE stream_shuffle moves them onto their head's
partitions and one fast-reciprocal + two multiplies normalize Y^T.
"""

import os
import sys

import numpy as np

for _p in ("/opt/trn_rl_repo", "/root/.axon_site/_ro/trn_rl_repo"):
    if os.path.isdir(_p) and _p not in sys.path:
        sys.path.append(_p)

B, T, E, H, D = 2, 2048, 1024, 16, 64
NC_CORES = 8
TT = B * T            # 4096 total tokens
EC = E // 128         # 8 contraction chunks
TG = TT // 512        # 8 token groups for the QKV pass
QC = T // 512         # 4 query chunks per batch
KB = T // 128         # 16 key blocks per batch

_last_results = None   # test harness peeks at this for exec_time_ns


def _build_bass(mm_dtype="float32r"):
    import concourse.bacc as bacc
    import concourse.mybir as mybir
    import concourse.tile as tile
    from concourse.masks import make_identity

    FP = mybir.dt.float32
    # Dtype of every tensor feeding a matmul. float32r = fp32 storage with
    # PE-rounded (tf32-like) multiplies at ~4x the fp32 matmul rate; walrus
    # requires producers of matmul operands to round to it, so the whole feed
    # path is tagged. float32 gives exact (slow) matmuls with the same graph.
    MF = getattr(mybir.dt, mm_dtype)
    AF = mybir.ActivationFunctionType

    nc = bacc.Bacc("TRN2", debug=False)
    xt = nc.declare_dram_parameter("xt", [EC, 128, TT], MF, isOutput=False)
    wqkv = nc.declare_dram_parameter("wqkv", [128, EC, 384], MF, isOutput=False)
    bqkv = nc.declare_dram_parameter("bqkv", [128, 3], FP, isOutput=False)
    wproj = nc.declare_dram_parameter("wproj", [128, E], MF, isOutput=False)
    y = nc.declare_dram_parameter("y", [TT, E], FP, isOutput=True)

    with tile.TileContext(nc) as tc:
        with tc.tile_pool(name="big", bufs=1) as big, \
             tc.tile_pool(name="const", bufs=1) as const:
            # weights/bias on the gpsimd DMA queue so the x-chunk loads own
            # the sync queue from instruction 0
            w_sb = const.tile([128, EC, 384], MF, tag="w_sb")
            for e in range(EC):
                nc.gpsimd.dma_start(out=w_sb[:, e, :], in_=wqkv[:, e, :])
            b_sb = const.tile([128, 3], FP, tag="b_sb")
            nc.gpsimd.dma_start(out=b_sb, in_=bqkv[:, :])
            wp_sb = const.tile([128, E], MF, tag="wp_sb")
            nc.gpsimd.dma_start(out=wp_sb, in_=wproj[:, :])
            ident32 = const.tile([128, 128], FP, tag="ident32")
            make_identity(nc, ident32)
            ident = const.tile([128, 128], MF, tag="ident")
            nc.vector.tensor_copy(ident, ident32)
            # fp32 image of one V_aug block's cols 64:192 (all denominator ones)
            vpat = const.tile([128, 128], FP, tag="vpat")
            nc.gpsimd.memset(vpat, 1.0)
            # tri[k, q] = 1.0 where q >= k else 0.0  (keep-mask applied after exp)
            trimask = const.tile([128, 128], FP, tag="trimask")
            nc.gpsimd.memset(trimask, 1.0)
            nc.gpsimd.affine_select(
                out=trimask, in_=trimask,
                compare_op=mybir.AluOpType.is_ge,
                fill=0.0, base=0,
                pattern=[[1, 128]], channel_multiplier=-1,
            )

            qt_sb = big.tile([128, TT], MF, tag="qt")    # [c(2 heads x 64d), t]
            kt_sb = big.tile([128, TT], MF, tag="kt")
            vt_sb = big.tile([128, TT], MF, tag="vt")
            # V_aug blocks, per 128-token block j:
            #   cols   0: 64  V_h0      cols  64:128 ones
            #   cols 128:192 ones       cols 192:256 V_h1
            # lhsT h0 = [:, j, 0:128]  -> psum rows 0:64 Y_h0, rows 64:128 denom_h0
            # lhsT h1 = [:, j, 128:256]-> psum rows 0:64 denom_h1, rows 64:128 Y_h1
            vaug = big.tile([128, KB * B, 256], MF, tag="vaug")
            yt_sb = big.tile([128, TT], MF, tag="yt")

            # ---- Phase 1: Q^T / K^T / V^T = W.T @ x^T (+bias), with the
            # V-block PE-transposes interleaved per token group (a standalone
            # transpose pass doesn't count as PE-busy for the HAM clock gate
            # and lets it throttle right before the attention phase) ----
            with tc.tile_pool(name="xin", bufs=8) as xin, \
                 tc.tile_pool(name="qkv_ps", bufs=2, space="PSUM") as qkv_ps, \
                 tc.tile_pool(name="tp_ps", bufs=2, space="PSUM") as tp_ps:
                for tg in range(TG):
                    ts = slice(tg * 512, (tg + 1) * 512)
                    pq = qkv_ps.tile([128, 512], FP, tag="pq")
                    pk = qkv_ps.tile([128, 512], FP, tag="pk")
                    pv = qkv_ps.tile([128, 512], FP, tag="pv")
                    for e in range(EC):
                        xc = xin.tile([128, 512], MF, tag="xc")
                        nc.sync.dma_start(out=xc, in_=xt[e, :, ts])
                        st_, sp_ = (e == 0), (e == EC - 1)
                        nc.tensor.matmul(pq, w_sb[:, e, 0:128], xc, start=st_, stop=sp_)
                        nc.tensor.matmul(pk, w_sb[:, e, 128:256], xc, start=st_, stop=sp_)
                        nc.tensor.matmul(pv, w_sb[:, e, 256:384], xc, start=st_, stop=sp_)
                    nc.scalar.activation(qt_sb[:, ts], pq, AF.Identity, bias=b_sb[:, 0:1])
                    nc.scalar.activation(kt_sb[:, ts], pk, AF.Identity, bias=b_sb[:, 1:2])
                    nc.scalar.activation(vt_sb[:, ts], pv, AF.Identity, bias=b_sb[:, 2:3])
                    if tg:  # transpose the PREVIOUS group's V blocks (hides
                        # the ACT->PE latency behind this group's matmuls)
                        for j in range(4 * (tg - 1), 4 * tg):
                            tp = tp_ps.tile([128, 128], MF, tag="tp")
                            nc.tensor.transpose(tp, vt_sb[:, j * 128:(j + 1) * 128], ident)
                            nc.vector.tensor_copy(vaug[:, j, 64:192], vpat)
                            nc.vector.tensor_copy(vaug[:, j, 0:64], tp[:, 0:64])
                            nc.vector.tensor_copy(vaug[:, j, 192:256], tp[:, 64:128])
                for j in range(4 * (TG - 1), 4 * TG):
                    tp = tp_ps.tile([128, 128], MF, tag="tp")
                    nc.tensor.transpose(tp, vt_sb[:, j * 128:(j + 1) * 128], ident)
                    nc.vector.tensor_copy(vaug[:, j, 64:192], vpat)
                    nc.vector.tensor_copy(vaug[:, j, 0:64], tp[:, 0:64])
                    nc.vector.tensor_copy(vaug[:, j, 192:256], tp[:, 64:128])

            # ---- Phase 2: attention, per batch, both heads ----
            def proj_block(pool, opool, tb):
                """Emit the output projection for one 128-token block."""
                for ecol in range(E // 512):
                    pp = pool.tile([128, 512], FP, tag="pp")
                    nc.tensor.matmul(
                        pp, yt_sb[:, tb * 128:(tb + 1) * 128],
                        wp_sb[:, ecol * 512:(ecol + 1) * 512],
                        start=True, stop=True)
                    ot = opool.tile([128, 512], FP, tag="ot")
                    nc.vector.tensor_copy(ot, pp)
                    nc.sync.dma_start(
                        out=y[tb * 128:(tb + 1) * 128, ecol * 512:(ecol + 1) * 512],
                        in_=ot)

            with tc.tile_pool(name="att_ps", bufs=1, space="PSUM") as att_ps, \
                 tc.tile_pool(name="st_ps", bufs=2, space="PSUM") as st_ps, \
                 tc.tile_pool(name="pj_ps", bufs=2, space="PSUM") as pj_ps, \
                 tc.tile_pool(name="pt_sb", bufs=4) as pt_pool, \
                 tc.tile_pool(name="out_sb", bufs=4) as out_pool, \
                 tc.tile_pool(name="rc_sb", bufs=2) as rc_pool:
                prev_tbs = []
                for b in range(B):
                    for qc in range(QC):
                        yt0 = att_ps.tile([128, 512], FP, tag="yt0", bufs=1)
                        yt1 = att_ps.tile([128, 512], FP, tag="yt1", bufs=1)
                        njb = 4 * qc + 4
                        for j in range(njb):
                            qoff = max(512 * qc, 128 * j)
                            width = 512 * qc + 512 - qoff
                            st = st_ps.tile([128, 2, 512], FP, tag="st")
                            q0 = b * T + qoff
                            k0 = b * T + 128 * j
                            nc.tensor.matmul(
                                st[:, 0, 0:width],
                                kt_sb[0:64, k0:k0 + 128],
                                qt_sb[0:64, q0:q0 + width],
                                start=True, stop=True)
                            nc.tensor.matmul(
                                st[:, 1, 0:width],
                                kt_sb[64:128, k0:k0 + 128],
                                qt_sb[64:128, q0:q0 + width],
                                start=True, stop=True)
                            pt = pt_pool.tile([128, 2, 512], MF, tag="pt")
                            nc.scalar.activation(
                                pt[:, :, 0:width], st[:, :, 0:width],
                                AF.Exp, scale=0.125)
                            if j >= 4 * qc:  # diagonal block: causal mask
                                nc.vector.tensor_mul(pt[:, 0, 0:128], pt[:, 0, 0:128], trimask)
                                nc.vector.tensor_mul(pt[:, 1, 0:128], pt[:, 1, 0:128], trimask)
                            off = qoff - 512 * qc
                            jb = b * KB + j
                            st_, sp_ = (j == 0), (j == njb - 1)
                            nc.tensor.matmul(
                                yt0[:, off:512], vaug[:, jb, 0:128],
                                pt[:, 0, 0:width], start=st_, stop=sp_)
                            nc.tensor.matmul(
                                yt1[:, off:512], vaug[:, jb, 128:256],
                                pt[:, 1, 0:width], start=st_, stop=sp_)
                        # normalize: yt0 = [Y_h0 (0:64) ; denom_h0 rep (64:128)]
                        #            yt1 = [denom_h1 rep (0:64) ; Y_h1 (64:128)]
                        # stream_shuffle moves the denom halves onto the same
                        # partitions as their Y half, then one recip + two muls.
                        iden_mask = list(range(32))
                        den_sb = rc_pool.tile([128, 512], FP, tag="den_sb")
                        nc.vector.stream_shuffle(den_sb[0:64, :], yt0[64:128, :], iden_mask)
                        nc.vector.stream_shuffle(den_sb[64:128, :], yt1[0:64, :], iden_mask)
                        rcb_sb = rc_pool.tile([128, 512], FP, tag="rcb_sb")
                        rscr = rc_pool.tile([128, 512], FP, tag="rscr")
                        nc.vector.reciprocal_approx_accurate(rcb_sb, den_sb, rscr)
                        cs = slice(b * T + 512 * qc, b * T + 512 * qc + 512)
                        nc.vector.tensor_mul(yt_sb[0:64, cs], yt0[0:64, :], rcb_sb[0:64, :])
                        nc.vector.tensor_mul(yt_sb[64:128, cs], yt1[64:128, :], rcb_sb[64:128, :])
                        # Emit the PREVIOUS q-chunk's output projection here:
                        # its deps are long satisfied, so these matmuls fill
                        # the PE gap while this chunk's normalization runs.
                        for tb in prev_tbs:
                            proj_block(pj_ps, out_pool, tb)
                        g = b * QC + qc
                        prev_tbs = list(range(4 * g, 4 * g + 4))
                for tb in prev_tbs:
                    proj_block(pj_ps, out_pool, tb)

    nc.compile()
    return nc


_nc_cache = {}


def get_nc(mm_dtype="float32r"):
    if mm_dtype not in _nc_cache:
        _nc_cache[mm_dtype] = _build_bass(mm_dtype)
    return _nc_cache[mm_dtype]


def make_in_maps(x, W_attn, b_attn, W_proj, feed_dtype=np.float32):
    x = np.ascontiguousarray(np.asarray(x, np.float32))
    W_attn = np.asarray(W_attn, np.float32)
    b_attn = np.asarray(b_attn, np.float32)
    W_proj = np.asarray(W_proj, np.float32)
    xtf = np.ascontiguousarray(x.reshape(TT, E).T.astype(feed_dtype)).reshape(EC, 128, TT)
    in_maps = []
    for c in range(NC_CORES):
        s = slice(128 * c, 128 * c + 128)
        wq = W_attn[:, 128 * c:128 * c + 128]
        wk = W_attn[:, E + 128 * c:E + 128 * c + 128]
        wv = W_attn[:, 2 * E + 128 * c:2 * E + 128 * c + 128]
        wqkv = np.concatenate([wq, wk, wv], axis=1)          # [E, 384]
        wqkv = np.ascontiguousarray(
            wqkv.reshape(EC, 128, 384).transpose(1, 0, 2).astype(feed_dtype))
        bq = b_attn[128 * c:128 * c + 128]
        bk = b_attn[E + 128 * c:E + 128 * c + 128]
        bv = b_attn[2 * E + 128 * c:2 * E + 128 * c + 128]
        bqkv = np.ascontiguousarray(np.stack([bq, bk, bv], axis=1))  # [128, 3]
        wp = np.ascontiguousarray(W_proj[s, :].astype(feed_dtype))   # [128, E]
        in_maps.append({"xt": xtf, "wqkv": wqkv, "bqkv": bqkv, "wproj": wp})
    return in_maps


def _install_ntff_shim():
    """The image's antenv package lacks axon_hooks, which bass_utils imports
    when trace=True under axon. Install it at runtime (registering the same
    ctypes-based hook trn_agent_boot would have set)."""
    import types
    import antenv
    if hasattr(antenv, "axon_hooks"):
        return
    mod = types.ModuleType("antenv.axon_hooks")
    mod._hook = None
    mod.set_axon_ntff_profile_hook = lambda h: setattr(mod, "_hook", h)
    mod.get_axon_ntff_profile_hook = lambda: mod._hook
    sys.modules["antenv.axon_hooks"] = mod
    antenv.axon_hooks = mod
    try:
        from trn_agent_boot.trn_boot import _ntff_profile_via_ctypes
        mod._hook = _ntff_profile_via_ctypes("/opt/axon/libaxon_pjrt.so")
    except Exception as e:
        print("ntff shim: no hook:", e)
    # upload_artifacts reaches external blob storage; stub it out.
    from concourse import bass_utils as bu
    bu.upload_artifacts = lambda tmpdir: tmpdir


def kernel(x, W_attn, b_attn, W_proj, b_proj):
    global _last_results
    from concourse.bass_utils import run_bass_kernel_spmd

    trace_req = bool(int(os.environ.get("KERNEL_TRACE", "0")))
    if trace_req:
        _install_ntff_shim()
    mm_dtype = os.environ.get("KERNEL_MM_DTYPE", "float32r")
    if mm_dtype == "bfloat16":
        import ml_dtypes
        feed_dtype = ml_dtypes.bfloat16
    else:
        feed_dtype = np.float32
    nc = get_nc(mm_dtype)
    in_maps = make_in_maps(x, W_attn, b_attn, W_proj, feed_dtype)
    res = run_bass_kernel_spmd(
        nc, in_maps, core_ids=list(range(NC_CORES)), trace=trace_req)
    _last_results = res
    y = res.results[0]["y"].astype(np.float64)
    for r in res.results[1:]:
        y += r["y"]
    y += np.asarray(b_proj, np.float64)
    return y.astype(np.float32).reshape(B, T, E)
